# revision 11
# baseline (speedup 1.0000x reference)
"""Causal self-attention (B=4, T=2048, C=1024, H=16, rope) on 8 trn2 cores.

Sharding: data-parallel over B (4) x tensor-parallel over heads (2 groups of
8 heads). Core (b, g) computes its batch's Q/K/V for its 8 heads, the full
causal attention for those heads, and a partial output projection
(y_heads @ wp_cols.T). Host sums the two head-group partials per batch and
adds the output bias.

v2: fully software-pipelined emission. The V projection runs first (only the
first 4 time-tiles are needed to start attention), then the per-head-pair
attention streams are interleaved with the NEXT pair's Q/K projection + rope
as "filler" work, so the scalar engine's exp stream (the steady-state
bottleneck) starts ~25us into the kernel instead of after all projections.
Input DMAs use host-pretiled layouts with >=4KB contiguous descriptors.
Softmax denominators use the extra all-ones column in V (row 64 of the O^T
accumulation); reciprocal uses the fast approximate DVE op. The causal mask
multiply handles both heads of a pair in one strided op.
"""

import sys

if "/opt/trn_rl_repo" not in sys.path:
    sys.path.insert(0, "/opt/trn_rl_repo")

from contextlib import ExitStack

import numpy as np

import concourse.bass as bass
import concourse.mybir as mybir
from concourse import bacc
from concourse.bass_utils import run_bass_kernel_spmd
from concourse.tile import TileContext

B, T, C = 4, 2048, 1024
H = 16
D = 64
NCORES = 8
CL = C // 2  # per-core c_out (8 heads * 64)
HL = 8  # local heads
F = mybir.dt.float32
FR = mybir.dt.bfloat16  # matmul operand dtype

_NC_CACHE = {}


def _build_nc(with_bias: bool):
    KC = 9 if with_bias else 8  # c_in chunks of 128 (one extra for bias row)
    nc = bacc.Bacc("TRN2", debug=False, num_devices=NCORES)

    xT2 = nc.declare_dram_parameter("xT2", [128, KC * T], FR, isOutput=False).ap()
    wq2 = nc.declare_dram_parameter("wq2", [128, KC * CL], FR, isOutput=False).ap()
    wk2 = nc.declare_dram_parameter("wk2", [128, KC * CL], FR, isOutput=False).ap()
    wv2 = nc.declare_dram_parameter("wv2", [128, KC * CL], FR, isOutput=False).ap()
    wp2 = nc.declare_dram_parameter("wp2", [128, 4 * C], FR, isOutput=False).ap()
    ropeC = nc.declare_dram_parameter("ropeC", [128, T], FR, isOutput=False).ap()
    ropeS = nc.declare_dram_parameter("ropeS", [128, T], FR, isOutput=False).ap()
    dmask2 = nc.declare_dram_parameter("dmask2", [128, 256], FR, isOutput=False).ap()
    out = nc.declare_dram_parameter("out", [T, C], F, isOutput=True).ap()

    EXP = mybir.ActivationFunctionType.Exp
    scale = 1.0 / float(np.sqrt(D))

    with TileContext(nc) as tc:
        with ExitStack() as ctx:
            qk_pool = ctx.enter_context(tc.tile_pool(name="qk", bufs=1))
            v_pool = ctx.enter_context(tc.tile_pool(name="v", bufs=1))
            xw_pool = ctx.enter_context(tc.tile_pool(name="xw", bufs=1))
            tpool = ctx.enter_context(tc.tile_pool(name="t1", bufs=2))
            ppool = ctx.enter_context(tc.tile_pool(name="pt", bufs=3))
            yrawp = ctx.enter_context(tc.tile_pool(name="yraw", bufs=3))
            ytmpp = ctx.enter_context(tc.tile_pool(name="ytmp", bufs=2))
            ynp = ctx.enter_context(tc.tile_pool(name="yn", bufs=1))
            osbp = ctx.enter_context(tc.tile_pool(name="osb", bufs=3))
            dpool = ctx.enter_context(tc.tile_pool(name="dd", bufs=2))
            bcpool = ctx.enter_context(tc.tile_pool(name="bc", bufs=3))
            spool = ctx.enter_context(tc.tile_pool(name="sps", bufs=2, space="PSUM"))
            opool = ctx.enter_context(tc.tile_pool(name="ops", bufs=1, space="PSUM"))
            prpool = ctx.enter_context(tc.tile_pool(name="prs", bufs=2, space="PSUM"))

            qt_sb = [
                qk_pool.tile([128, T], FR, tag=f"qt{m}", name=f"qt{m}")
                for m in range(4)
            ]
            kt_sb = [
                qk_pool.tile([128, T], FR, tag=f"kt{m}", name=f"kt{m}")
                for m in range(4)
            ]
            vaug = [
                v_pool.tile([128, HL, D + 1], FR, tag=f"va{j}", name=f"va{j}")
                for j in range(16)
            ]

            x_sb = xw_pool.tile([128, KC, T], FR, tag="x", name="x")
            wq_sb = xw_pool.tile([128, KC, CL], FR, tag="wq", name="wq")
            wk_sb = xw_pool.tile([128, KC, CL], FR, tag="wk", name="wk")
            wv_sb = xw_pool.tile([128, KC, CL], FR, tag="wv", name="wv")
            wp_sb = xw_pool.tile([128, 4, C], FR, tag="wp", name="wp")
            rc_sb = xw_pool.tile([128, T], FR, tag="rc", name="rc")
            rs_sb = xw_pool.tile([128, T], FR, tag="rs", name="rs")
            dm_sb = xw_pool.tile([128, 256], FR, tag="dm", name="dm")

            # ---- input DMAs, need-ordered: V deps first ----
            nc.sync.dma_start(out=wv_sb, in_=wv2.rearrange("p (k m) -> p k m", k=KC))
            x_r = xT2.rearrange("p (k t) -> p k t", k=KC)
            for k in range(KC):
                nc.sync.dma_start(out=x_sb[:, k, :], in_=x_r[:, k, :])
            nc.sync.dma_start(out=wk_sb, in_=wk2.rearrange("p (k m) -> p k m", k=KC))
            nc.sync.dma_start(out=wq_sb, in_=wq2.rearrange("p (k m) -> p k m", k=KC))
            nc.sync.dma_start(out=rc_sb, in_=ropeC)
            nc.sync.dma_start(out=rs_sb, in_=ropeS)
            nc.sync.dma_start(out=dm_sb, in_=dmask2)
            nc.sync.dma_start(out=wp_sb, in_=wp2.rearrange("p (j n) -> p j n", j=4))

            # ones column of V-augmented tiles (softmax denominators)
            for j in range(16):
                nc.gpsimd.memset(vaug[j][:, :, D : D + 1], 1.0)

            # preload the exp table set during the input DMA window
            warm = tpool.tile([128, 512], FR, tag="qcp", name="warm")
            nc.gpsimd.memset(warm[0:1, 0:1], 0.0)
            nc.scalar.activation(warm[0:1, 1:2], warm[0:1, 0:1], EXP)

            # ---------------- filler work-unit generators ----------------
            def v_group(jj):
                ps = prpool.tile([128, 512], F, tag="pr", name="pr")
                for k in range(KC):
                    nc.tensor.matmul(
                        ps,
                        lhsT=x_sb[:, k, 128 * jj : 128 * (jj + 1)],
                        rhs=wv_sb[:, k, :],
                        start=(k == 0),
                        stop=(k == KC - 1),
                    )
                    if k == 3:
                        yield
                nc.vector.tensor_copy(
                    out=vaug[jj][:, :, 0:D],
                    in_=ps.rearrange("p (h d) -> p h d", h=HL),
                )
                yield

            def qk_group(m, which, t):
                wsb = wk_sb if which == "k" else wq_sb
                dst = kt_sb[m] if which == "k" else qt_sb[m]
                ps = prpool.tile([128, 512], F, tag="pr", name="pr")
                for k in range(KC):
                    nc.tensor.matmul(
                        ps,
                        lhsT=wsb[:, k, 128 * m : 128 * (m + 1)],
                        rhs=x_sb[:, k, 512 * t : 512 * (t + 1)],
                        start=(k == 0),
                        stop=(k == KC - 1),
                    )
                    if k == 3:
                        yield
                qcp = tpool.tile([128, 512], FR, tag="qcp", name="qcp")
                nc.vector.tensor_copy(qcp, ps)
                qsw = tpool.tile([128, 512], FR, tag="qsw", name="qsw")
                for a, b in ((0, 32), (32, 0), (64, 96), (96, 64)):
                    nc.sync.dma_start(out=qsw[a : a + 32, :], in_=qcp[b : b + 32, :])
                yield
                t1 = tpool.tile([128, 512], FR, tag="t1", name="t1")
                t2 = tpool.tile([128, 512], FR, tag="t2", name="t2")
                nc.gpsimd.tensor_mul(t1, qcp, rc_sb[:, 512 * t : 512 * (t + 1)])
                nc.vector.tensor_mul(t2, qsw, rs_sb[:, 512 * t : 512 * (t + 1)])
                nc.vector.tensor_add(dst[:, 512 * t : 512 * (t + 1)], t1, t2)
                yield

            def outproj_group(ci, yn):
                for g in range(8):
                    tt, cc = g % 4, g // 4
                    pr = prpool.tile([128, 512], F, tag="pr", name="pr")
                    for p4 in range(4):
                        nc.tensor.matmul(
                            pr,
                            lhsT=yn[p4][:, 128 * tt : 128 * (tt + 1)],
                            rhs=wp_sb[:, p4, 512 * cc : 512 * (cc + 1)],
                            start=(p4 == 0),
                            stop=(p4 == 3),
                        )
                    osb = osbp.tile([128, 512], F, tag="osb", name="osb")
                    nc.vector.tensor_copy(osb, pr)
                    nc.sync.dma_start(
                        out=out[
                            512 * ci + 128 * tt : 512 * ci + 128 * (tt + 1),
                            512 * cc : 512 * (cc + 1),
                        ],
                        in_=osb,
                    )
                    yield

            # ordered filler queue: (key, generator); ensure() drains in order
            fill_q = []
            fill_done = set()

            def push(key, gen):
                fill_q.append((key, gen))

            def ensure(key):
                while key not in fill_done and fill_q:
                    k0, g0 = fill_q[0]
                    try:
                        next(g0)
                    except StopIteration:
                        fill_done.add(k0)
                        fill_q.pop(0)

            def pump(n):
                for _ in range(n):
                    if not fill_q:
                        return
                    k0, g0 = fill_q[0]
                    try:
                        next(g0)
                    except StopIteration:
                        fill_done.add(k0)
                        fill_q.pop(0)

            # eager prologue: V tiles 0-3 and the (m=0, t=0) Q/K chunks
            for jj in range(4):
                g = v_group(jj)
                for _ in g:
                    pass
                fill_done.add(("v", jj))
            for which in ("k", "q"):
                g = qk_group(0, which, 0)
                for _ in g:
                    pass
                fill_done.add((which, 0, 0))

            # filler queue in need-order for the p=0 attention sweep, then
            # the later pairs' projections
            for ci in range(1, 4):
                for jj in range(4 * ci, 4 * ci + 4):
                    push(("v", jj), v_group(jj))
                push(("k", 0, ci), qk_group(0, "k", ci))
                push(("q", 0, ci), qk_group(0, "q", ci))
            for m in range(1, 4):
                for t in range(4):
                    push(("k", m, t), qk_group(m, "k", t))
                    push(("q", m, t), qk_group(m, "q", t))

            # ---------------- attention sweeps, pair-major ----------------
            yn_store = {}  # ci -> [yn tiles p=0..3]
            pending_norm = None
            outproj_ready = []  # list of ci whose outproj can be pushed

            for p in range(4):
                for ci in range(4):
                    ensure(("v", 4 * ci + 3))
                    ensure(("k", p, ci))
                    ensure(("q", p, ci))
                    yn = yn_store.setdefault(ci, [])
                    o_ps = [
                        opool.tile([128, 512], F, tag=f"o{h}", name=f"o{h}")
                        for h in range(2)
                    ]
                    ntj = 4 * ci + 4
                    for tj in range(ntj):
                        kk = tj - 4 * ci
                        off = 128 * max(kk, 0)
                        s_ps = spool.tile([128, 1024], F, tag="s", name="s")
                        for h in range(2):
                            nc.tensor.matmul(
                                s_ps[:, 512 * h + off : 512 * h + 512],
                                lhsT=kt_sb[p][
                                    64 * h : 64 * h + 64,
                                    128 * tj : 128 * (tj + 1),
                                ],
                                rhs=qt_sb[p][
                                    64 * h : 64 * h + 64,
                                    512 * ci + off : 512 * (ci + 1),
                                ],
                                start=True,
                                stop=True,
                                tile_position=(64 * h, 0),
                            )
                        pt = ppool.tile([128, 1024], FR, tag="pt", name="pt")
                        if kk < 0:
                            nc.scalar.activation(pt, s_ps, EXP, scale=scale)
                        else:
                            s_v = s_ps.rearrange("q (h n) -> q h n", h=2)[:, :, off:]
                            p_v = pt.rearrange("q (h n) -> q h n", h=2)[:, :, off:]
                            nc.scalar.activation(p_v, s_v, EXP, scale=scale)
                            # multiplicative causal mask on both diagonal blocks
                            blk = pt.rearrange("q (h n) -> q h n", h=2)[
                                :, :, off : off + 128
                            ]
                            nc.gpsimd.tensor_mul(
                                blk, blk, dm_sb.rearrange("q (h n) -> q h n", h=2)
                            )
                        for h in range(2):
                            nc.tensor.matmul(
                                o_ps[h][0 : D + 1, off:512],
                                lhsT=vaug[tj][:, 2 * p + h, :],
                                rhs=pt[:, 512 * h + off : 512 * h + 512],
                                start=(tj == 0),
                                stop=(tj == ntj - 1),
                                skip_group_check=True,
                            )
                        pump(1)

                    # epilogue: extract O + denominators, normalize (deferred)
                    yraw = yrawp.tile([128, 512], F, tag="yraw", name="yraw")
                    ytmp = ytmpp.tile([128, 512], F, tag="ytmp", name="ytmp")
                    d_sb = dpool.tile([128, 1024], F, tag="D", name="D")
                    nc.vector.tensor_copy(yraw[0:65, :], o_ps[0][0:65, :])
                    nc.vector.tensor_copy(ytmp[0:65, :], o_ps[1][0:65, :])
                    nc.gpsimd.dma_start(out=d_sb[0:1, 0:512], in_=yraw[64:65, :])
                    nc.gpsimd.dma_start(out=d_sb[1:2, 0:512], in_=ytmp[64:65, :])
                    nc.gpsimd.dma_start(out=yraw[64:128, :], in_=ytmp[0:64, :])
                    nc.vector.reciprocal_approx_fast(
                        out=d_sb[0:2, 512:1024], in_=d_sb[0:2, 0:512]
                    )
                    bc = bcpool.tile([128, 512], F, tag="bc", name="bc")
                    for h, eng in ((0, nc.gpsimd), (1, nc.gpsimd)):
                        sl = d_sb[h : h + 1, 512:1024]
                        bsrc = bass.AP(
                            tensor=sl.tensor,
                            offset=sl.offset,
                            ap=[list(sl.ap[0]), [0, 64], [1, 512]],
                        )
                        eng.dma_start(out=bc[64 * h : 64 * h + 64, :], in_=bsrc)
                    if pending_norm is not None:
                        pyn, pyraw, pbc, pci, pp = pending_norm
                        pynorm = ynp.tile([128, 512], FR, tag=f"yn{pci}_{pp}",
                                          name=f"yn{pci}_{pp}")
                        nc.vector.tensor_mul(pynorm, pyraw, pbc)
                        pyn.append(pynorm)
                        if pp == 3:
                            outproj_ready.append(pci)
                    pending_norm = (yn, yraw, bc, ci, p)
                    # during the last pair, stream out-projection as filler
                    while outproj_ready:
                        pci = outproj_ready.pop(0)
                        push(("op", pci), outproj_group(pci, yn_store[pci]))

            # tail: final deferred norm, remaining out-projection
            pyn, pyraw, pbc, pci, pp = pending_norm
            pynorm = ynp.tile([128, 512], FR, tag=f"yn{pci}_{pp}",
                              name=f"yn{pci}_{pp}")
            nc.vector.tensor_mul(pynorm, pyraw, pbc)
            pyn.append(pynorm)
            outproj_ready.append(pci)
            while outproj_ready:
                pci2 = outproj_ready.pop(0)
                push(("op", pci2), outproj_group(pci2, yn_store[pci2]))
            pump(10**6)

    nc.compile()
    return nc


def _get_nc(with_bias: bool):
    if with_bias not in _NC_CACHE:
        _NC_CACHE[with_bias] = _build_nc(with_bias)
    return _NC_CACHE[with_bias]


def _rope_tables():
    half = D // 2
    i = np.arange(half, dtype=np.float32)
    expo = (2.0 * i / np.float32(D)).astype(np.float32)
    alpha = (1.0 / (np.float32(10000.0) ** expo)).astype(np.float32)
    ang = (np.arange(T, dtype=np.float32)[:, None] * alpha[None, :]).astype(np.float32)
    cosv = np.cos(ang).astype(np.float32).T  # [32, T]
    sinv = np.sin(ang).astype(np.float32).T
    c64 = np.concatenate([cosv, cosv], axis=0)  # [64, T]
    s64 = np.concatenate([-sinv, sinv], axis=0)
    ropeC = np.ascontiguousarray(np.concatenate([c64, c64], axis=0))  # [128, T]
    ropeS = np.ascontiguousarray(np.concatenate([s64, s64], axis=0))
    import ml_dtypes

    return ropeC.astype(ml_dtypes.bfloat16), ropeS.astype(ml_dtypes.bfloat16)


import ml_dtypes


def _round_fp32r(a):
    """Cast host data to the matmul operand dtype (bf16)."""
    return np.ascontiguousarray(
        np.asarray(a, dtype=np.float32).astype(ml_dtypes.bfloat16)
    )


def _tile128(a, KC):
    """[KC*128, N] -> [128, KC*N] with per-partition contiguous rows."""
    N = a.shape[1]
    return np.ascontiguousarray(
        a.reshape(KC, 128, N).transpose(1, 0, 2).reshape(128, KC * N)
    )


def _make_in_maps(x, wq, bq, wk, bk, wv, bv, wp, with_bias):
    ropeC, ropeS = _rope_tables()
    dmask = np.triu(np.ones((128, 128), np.float32))
    dmask2 = np.ascontiguousarray(
        np.concatenate([dmask, dmask], axis=1).astype(ml_dtypes.bfloat16)
    )
    KC = 9 if with_bias else 8
    in_maps = []
    for b in range(B):
        xb = np.ascontiguousarray(x[b].T.astype(np.float32, copy=False))  # [C, T]
        if with_bias:
            aug = np.zeros((9 * 128 - C, T), np.float32)
            aug[0, :] = 1.0
            xb = np.concatenate([xb, aug], axis=0)
        xT2 = _tile128(_round_fp32r(xb), KC)
        for g in range(2):
            sl = slice(g * CL, (g + 1) * CL)
            wqTc = np.ascontiguousarray(wq[sl, :].T.astype(np.float32, copy=False))
            wkTc = np.ascontiguousarray(wk[sl, :].T.astype(np.float32, copy=False))
            wvTc = np.ascontiguousarray(wv[sl, :].T.astype(np.float32, copy=False))
            if with_bias:
                npad = 9 * 128 - C

                def _aug_w(wT, bias):
                    a = np.zeros((npad, CL), np.float32)
                    a[0, :] = bias[sl].astype(np.float32, copy=False)
                    return np.ascontiguousarray(np.concatenate([wT, a], axis=0))

                wqTc = _aug_w(wqTc, bq)
                wkTc = _aug_w(wkTc, bk)
                wvTc = _aug_w(wvTc, bv)
            wpTc = np.ascontiguousarray(wp[:, sl].T.astype(np.float32, copy=False))
            in_maps.append(
                {
                    "xT2": xT2,
                    "wq2": _tile128(_round_fp32r(wqTc), KC),
                    "wk2": _tile128(_round_fp32r(wkTc), KC),
                    "wv2": _tile128(_round_fp32r(wvTc), KC),
                    "wp2": _tile128(_round_fp32r(wpTc), 4),
                    "ropeC": ropeC,
                    "ropeS": ropeS,
                    "dmask2": dmask2,
                }
            )
    return in_maps


def _gather(results, bp):
    out = np.empty((B, T, C), dtype=np.float32)
    bp32 = np.asarray(bp, dtype=np.float32)
    for b in range(B):
        out[b] = results[2 * b]["out"] + results[2 * b + 1]["out"] + bp32
    return out


def run(x, wq, bq, wk, bk, wv, bv, wp, bp, trace=False, **kw):
    """Build/compile (cached), run on 8 cores, gather. Returns (out, results)."""
    arrs = [np.asarray(a) for a in (x, wq, bq, wk, bk, wv, bv, wp, bp)]
    x, wq, bq, wk, bk, wv, bv, wp, bp = arrs
    with_bias = bool(np.any(bq) or np.any(bk) or np.any(bv))
    nc = _get_nc(with_bias)
    in_maps = _make_in_maps(x, wq, bq, wk, bk, wv, bv, wp, with_bias)
    res = run_bass_kernel_spmd(nc, in_maps, list(range(NCORES)), trace=trace, **kw)
    return _gather(res.results, bp), res


def kernel(x, wq, bq, wk, bk, wv, bv, wp, bp):
    out, _ = run(x, wq, bq, wk, bk, wv, bv, wp, bp)
    return out


# revision 14
# speedup vs baseline: 1.0029x; 1.0029x over previous
"""Causal self-attention (B=4, T=2048, C=1024, H=16, rope) on 8 trn2 cores.

Sharding: data-parallel over B (4) x tensor-parallel over heads (2 groups of
8 heads). Core (b, g) computes its batch's Q/K/V for its 8 heads, the full
causal attention for those heads, and a partial output projection
(y_heads @ wp_cols.T). Host sums the two head-group partials per batch and
adds the output bias.

v2: fully software-pipelined emission. The V projection runs first (only the
first 4 time-tiles are needed to start attention), then the per-head-pair
attention streams are interleaved with the NEXT pair's Q/K projection + rope
as "filler" work, so the scalar engine's exp stream (the steady-state
bottleneck) starts ~25us into the kernel instead of after all projections.
Input DMAs use host-pretiled layouts with >=4KB contiguous descriptors.
Softmax denominators use the extra all-ones column in V (row 64 of the O^T
accumulation); reciprocal uses the fast approximate DVE op. The causal mask
multiply handles both heads of a pair in one strided op.
"""

import sys

if "/opt/trn_rl_repo" not in sys.path:
    sys.path.insert(0, "/opt/trn_rl_repo")

from contextlib import ExitStack

import numpy as np

import concourse.bass as bass
import concourse.mybir as mybir
from concourse import bacc
from concourse.bass_utils import run_bass_kernel_spmd
from concourse.tile import TileContext

B, T, C = 4, 2048, 1024
H = 16
D = 64
NCORES = 8
CL = C // 2  # per-core c_out (8 heads * 64)
HL = 8  # local heads
F = mybir.dt.float32
FR = mybir.dt.bfloat16  # matmul operand dtype

_NC_CACHE = {}


def _build_nc(with_bias: bool):
    KC = 9 if with_bias else 8  # c_in chunks of 128 (one extra for bias row)
    nc = bacc.Bacc("TRN2", debug=False, num_devices=NCORES)

    xT2 = nc.declare_dram_parameter("xT2", [128, KC * T], FR, isOutput=False).ap()
    wq2 = nc.declare_dram_parameter("wq2", [128, KC * CL], FR, isOutput=False).ap()
    wk2 = nc.declare_dram_parameter("wk2", [128, KC * CL], FR, isOutput=False).ap()
    wv2 = nc.declare_dram_parameter("wv2", [128, KC * CL], FR, isOutput=False).ap()
    wp2 = nc.declare_dram_parameter("wp2", [128, 4 * C], FR, isOutput=False).ap()
    ropeC = nc.declare_dram_parameter("ropeC", [128, T], FR, isOutput=False).ap()
    ropeS = nc.declare_dram_parameter("ropeS", [128, T], FR, isOutput=False).ap()
    dmask2 = nc.declare_dram_parameter("dmask2", [128, 256], FR, isOutput=False).ap()
    out = nc.declare_dram_parameter("out", [T, C], F, isOutput=True).ap()

    EXP = mybir.ActivationFunctionType.Exp
    scale = 1.0 / float(np.sqrt(D))

    with TileContext(nc) as tc:
        with ExitStack() as ctx:
            qk_pool = ctx.enter_context(tc.tile_pool(name="qk", bufs=1))
            v_pool = ctx.enter_context(tc.tile_pool(name="v", bufs=1))
            xw_pool = ctx.enter_context(tc.tile_pool(name="xw", bufs=1))
            tpool = ctx.enter_context(tc.tile_pool(name="t1", bufs=2))
            ppool = ctx.enter_context(tc.tile_pool(name="pt", bufs=3))
            yrawp = ctx.enter_context(tc.tile_pool(name="yraw", bufs=3))
            ytmpp = ctx.enter_context(tc.tile_pool(name="ytmp", bufs=2))
            ynp = ctx.enter_context(tc.tile_pool(name="yn", bufs=1))
            osbp = ctx.enter_context(tc.tile_pool(name="osb", bufs=3))
            dpool = ctx.enter_context(tc.tile_pool(name="dd", bufs=2))
            bcpool = ctx.enter_context(tc.tile_pool(name="bc", bufs=3))
            spool = ctx.enter_context(tc.tile_pool(name="sps", bufs=2, space="PSUM"))
            opool = ctx.enter_context(tc.tile_pool(name="ops", bufs=1, space="PSUM"))
            prpool = ctx.enter_context(tc.tile_pool(name="prs", bufs=2, space="PSUM"))

            qt_sb = [
                qk_pool.tile([128, T], FR, tag=f"qt{m}", name=f"qt{m}")
                for m in range(4)
            ]
            kt_sb = [
                qk_pool.tile([128, T], FR, tag=f"kt{m}", name=f"kt{m}")
                for m in range(4)
            ]
            vaug = [
                v_pool.tile([128, HL, D + 1], FR, tag=f"va{j}", name=f"va{j}")
                for j in range(16)
            ]

            x_sb = xw_pool.tile([128, KC, T], FR, tag="x", name="x")
            wq_sb = xw_pool.tile([128, KC, CL], FR, tag="wq", name="wq")
            wk_sb = xw_pool.tile([128, KC, CL], FR, tag="wk", name="wk")
            wv_sb = xw_pool.tile([128, KC, CL], FR, tag="wv", name="wv")
            wp_sb = xw_pool.tile([128, 4, C], FR, tag="wp", name="wp")
            rc_sb = xw_pool.tile([128, T], FR, tag="rc", name="rc")
            rs_sb = xw_pool.tile([128, T], FR, tag="rs", name="rs")
            dm_sb = xw_pool.tile([128, 256], FR, tag="dm", name="dm")

            # ---- input DMAs, need-ordered: V deps first ----
            nc.sync.dma_start(out=wv_sb, in_=wv2.rearrange("p (k m) -> p k m", k=KC))
            x_r = xT2.rearrange("p (k t) -> p k t", k=KC)
            for k in range(KC):
                nc.sync.dma_start(out=x_sb[:, k, :], in_=x_r[:, k, :])
            nc.sync.dma_start(out=wk_sb, in_=wk2.rearrange("p (k m) -> p k m", k=KC))
            nc.sync.dma_start(out=wq_sb, in_=wq2.rearrange("p (k m) -> p k m", k=KC))
            nc.sync.dma_start(out=rc_sb, in_=ropeC)
            nc.sync.dma_start(out=rs_sb, in_=ropeS)
            nc.sync.dma_start(out=dm_sb, in_=dmask2)
            nc.sync.dma_start(out=wp_sb, in_=wp2.rearrange("p (j n) -> p j n", j=4))

            # ones column of V-augmented tiles (softmax denominators)
            for j in range(16):
                nc.gpsimd.memset(vaug[j][:, :, D : D + 1], 1.0)

            # preload the exp table set during the input DMA window
            warm = tpool.tile([128, 512], FR, tag="qcp", name="warm")
            nc.gpsimd.memset(warm[0:1, 0:1], 0.0)
            nc.scalar.activation(warm[0:1, 1:2], warm[0:1, 0:1], EXP)

            # ---------------- filler work-unit generators ----------------
            def v_group(jj):
                ps = prpool.tile([128, 512], F, tag="pr", name="pr")
                for k in range(KC):
                    nc.tensor.matmul(
                        ps,
                        lhsT=x_sb[:, k, 128 * jj : 128 * (jj + 1)],
                        rhs=wv_sb[:, k, :],
                        start=(k == 0),
                        stop=(k == KC - 1),
                    )
                    if k == 3:
                        yield
                nc.vector.tensor_copy(
                    out=vaug[jj][:, :, 0:D],
                    in_=ps.rearrange("p (h d) -> p h d", h=HL),
                )
                yield

            def qk_group(m, which, t):
                wsb = wk_sb if which == "k" else wq_sb
                dst = kt_sb[m] if which == "k" else qt_sb[m]
                ps = prpool.tile([128, 512], F, tag="pr", name="pr")
                for k in range(KC):
                    nc.tensor.matmul(
                        ps,
                        lhsT=wsb[:, k, 128 * m : 128 * (m + 1)],
                        rhs=x_sb[:, k, 512 * t : 512 * (t + 1)],
                        start=(k == 0),
                        stop=(k == KC - 1),
                    )
                    if k == 3:
                        yield
                qcp = tpool.tile([128, 512], FR, tag="qcp", name="qcp")
                nc.vector.tensor_copy(qcp, ps)
                qsw = tpool.tile([128, 512], FR, tag="qsw", name="qsw")
                for a, b in ((0, 32), (32, 0), (64, 96), (96, 64)):
                    nc.sync.dma_start(out=qsw[a : a + 32, :], in_=qcp[b : b + 32, :])
                yield
                t1 = tpool.tile([128, 512], FR, tag="t1", name="t1")
                t2 = tpool.tile([128, 512], FR, tag="t2", name="t2")
                nc.gpsimd.tensor_mul(t1, qcp, rc_sb[:, 512 * t : 512 * (t + 1)])
                nc.vector.tensor_mul(t2, qsw, rs_sb[:, 512 * t : 512 * (t + 1)])
                nc.vector.tensor_add(dst[:, 512 * t : 512 * (t + 1)], t1, t2)
                yield

            def outproj_group(ci, yn):
                for g in range(8):
                    tt, cc = g % 4, g // 4
                    pr = prpool.tile([128, 512], F, tag="pr", name="pr")
                    for p4 in range(4):
                        nc.tensor.matmul(
                            pr,
                            lhsT=yn[p4][:, 128 * tt : 128 * (tt + 1)],
                            rhs=wp_sb[:, p4, 512 * cc : 512 * (cc + 1)],
                            start=(p4 == 0),
                            stop=(p4 == 3),
                        )
                    osb = osbp.tile([128, 512], F, tag="osb", name="osb")
                    nc.vector.tensor_copy(osb, pr)
                    nc.sync.dma_start(
                        out=out[
                            512 * ci + 128 * tt : 512 * ci + 128 * (tt + 1),
                            512 * cc : 512 * (cc + 1),
                        ],
                        in_=osb,
                    )
                    yield

            # ordered filler queue: (key, generator); ensure() drains in order
            fill_q = []
            fill_done = set()

            def push(key, gen):
                fill_q.append((key, gen))

            def ensure(key):
                while key not in fill_done and fill_q:
                    k0, g0 = fill_q[0]
                    try:
                        next(g0)
                    except StopIteration:
                        fill_done.add(k0)
                        fill_q.pop(0)

            def pump(n):
                for _ in range(n):
                    if not fill_q:
                        return
                    k0, g0 = fill_q[0]
                    try:
                        next(g0)
                    except StopIteration:
                        fill_done.add(k0)
                        fill_q.pop(0)

            # eager prologue: V tiles 0-3 and the (m=0, t=0) Q/K chunks
            for jj in range(4):
                g = v_group(jj)
                for _ in g:
                    pass
                fill_done.add(("v", jj))
            for which in ("k", "q"):
                g = qk_group(0, which, 0)
                for _ in g:
                    pass
                fill_done.add((which, 0, 0))

            # filler queue in need-order for the p=0 attention sweep, then
            # the later pairs' projections
            for ci in range(1, 4):
                for jj in range(4 * ci, 4 * ci + 4):
                    push(("v", jj), v_group(jj))
                push(("k", 0, ci), qk_group(0, "k", ci))
                push(("q", 0, ci), qk_group(0, "q", ci))
            for m in range(1, 4):
                for t in range(4):
                    push(("k", m, t), qk_group(m, "k", t))
                    push(("q", m, t), qk_group(m, "q", t))

            # ---------------- attention sweeps, pair-major ----------------
            yn_store = {}  # ci -> [yn tiles p=0..3]
            ep_q = []  # pipelined epilogue records [yn, yraw, d_sb, bc, ci, p]
            outproj_ready = []  # list of ci whose outproj can be pushed

            def flush_norm(e):
                pyn, pyraw, _, pbc, pci, pp = e
                pynorm = ynp.tile(
                    [128, 512], FR, tag=f"yn{pci}_{pp}", name=f"yn{pci}_{pp}"
                )
                nc.vector.tensor_mul(pynorm, pyraw, pbc)
                pyn.append(pynorm)
                if pp == 3:
                    outproj_ready.append(pci)

            for p in range(4):
                for ci in range(4):
                    ensure(("v", 4 * ci + 3))
                    ensure(("k", p, ci))
                    ensure(("q", p, ci))
                    yn = yn_store.setdefault(ci, [])
                    o_ps = [
                        opool.tile([128, 512], F, tag=f"o{h}", name=f"o{h}")
                        for h in range(2)
                    ]
                    ntj = 4 * ci + 4
                    for tj in range(ntj):
                        kk = tj - 4 * ci
                        off = 128 * max(kk, 0)
                        s_ps = spool.tile([128, 1024], F, tag="s", name="s")
                        for h in range(2):
                            nc.tensor.matmul(
                                s_ps[:, 512 * h + off : 512 * h + 512],
                                lhsT=kt_sb[p][
                                    64 * h : 64 * h + 64,
                                    128 * tj : 128 * (tj + 1),
                                ],
                                rhs=qt_sb[p][
                                    64 * h : 64 * h + 64,
                                    512 * ci + off : 512 * (ci + 1),
                                ],
                                start=True,
                                stop=True,
                                tile_position=(64 * h, 0),
                            )
                        pt = ppool.tile([128, 1024], FR, tag="pt", name="pt")
                        if kk < 0:
                            nc.scalar.activation(pt, s_ps, EXP, scale=scale)
                        else:
                            s_v = s_ps.rearrange("q (h n) -> q h n", h=2)[:, :, off:]
                            p_v = pt.rearrange("q (h n) -> q h n", h=2)[:, :, off:]
                            nc.scalar.activation(p_v, s_v, EXP, scale=scale)
                            # multiplicative causal mask on both diagonal blocks
                            blk = pt.rearrange("q (h n) -> q h n", h=2)[
                                :, :, off : off + 128
                            ]
                            nc.gpsimd.tensor_mul(
                                blk, blk, dm_sb.rearrange("q (h n) -> q h n", h=2)
                            )
                        for h in range(2):
                            nc.tensor.matmul(
                                o_ps[h][0 : D + 1, off:512],
                                lhsT=vaug[tj][:, 2 * p + h, :],
                                rhs=pt[:, 512 * h + off : 512 * h + 512],
                                start=(tj == 0),
                                stop=(tj == ntj - 1),
                                skip_group_check=True,
                            )
                        pump(1)

                    # epilogue: extract O + denominators. The normalization
                    # chain is pipelined 3 sweeps deep (copies+saves now,
                    # recip+broadcast next sweep, normalize the sweep after)
                    # so its DMA drains never gate the attention steady state.
                    yraw = yrawp.tile([128, 512], F, tag="yraw", name="yraw")
                    ytmp = ytmpp.tile([128, 512], F, tag="ytmp", name="ytmp")
                    d_sb = dpool.tile([128, 1024], F, tag="D", name="D")
                    nc.vector.tensor_copy(yraw[0:65, :], o_ps[0][0:65, :])
                    nc.vector.tensor_copy(ytmp[0:65, :], o_ps[1][0:65, :])
                    nc.gpsimd.dma_start(out=d_sb[0:1, 0:512], in_=yraw[64:65, :])
                    nc.gpsimd.dma_start(out=d_sb[1:2, 0:512], in_=ytmp[64:65, :])
                    nc.gpsimd.dma_start(out=yraw[64:128, :], in_=ytmp[0:64, :])
                    ep_q.append([yn, yraw, d_sb, None, ci, p])
                    if len(ep_q) >= 2:
                        e = ep_q[-2]
                        nc.vector.reciprocal_approx_fast(
                            out=e[2][0:2, 512:1024], in_=e[2][0:2, 0:512]
                        )
                        bc = bcpool.tile([128, 512], F, tag="bc", name="bc")
                        for h in range(2):
                            sl = e[2][h : h + 1, 512:1024]
                            bsrc = bass.AP(
                                tensor=sl.tensor,
                                offset=sl.offset,
                                ap=[list(sl.ap[0]), [0, 64], [1, 512]],
                            )
                            nc.gpsimd.dma_start(
                                out=bc[64 * h : 64 * h + 64, :], in_=bsrc
                            )
                        e[3] = bc
                    if len(ep_q) >= 3:
                        flush_norm(ep_q.pop(0))
                    # during the last pair, stream out-projection as filler
                    while outproj_ready:
                        pci = outproj_ready.pop(0)
                        push(("op", pci), outproj_group(pci, yn_store[pci]))

            # tail: flush the pipelined epilogues, remaining out-projection
            e = ep_q[-1]
            nc.vector.reciprocal_approx_fast(
                out=e[2][0:2, 512:1024], in_=e[2][0:2, 0:512]
            )
            bc = bcpool.tile([128, 512], F, tag="bc", name="bc")
            for h in range(2):
                sl = e[2][h : h + 1, 512:1024]
                bsrc = bass.AP(
                    tensor=sl.tensor,
                    offset=sl.offset,
                    ap=[list(sl.ap[0]), [0, 64], [1, 512]],
                )
                nc.gpsimd.dma_start(out=bc[64 * h : 64 * h + 64, :], in_=bsrc)
            e[3] = bc
            while ep_q:
                flush_norm(ep_q.pop(0))
                while outproj_ready:
                    pci2 = outproj_ready.pop(0)
                    push(("op", pci2), outproj_group(pci2, yn_store[pci2]))
                pump(10**6)
            pump(10**6)

    nc.compile()
    return nc


def _get_nc(with_bias: bool):
    if with_bias not in _NC_CACHE:
        _NC_CACHE[with_bias] = _build_nc(with_bias)
    return _NC_CACHE[with_bias]


def _rope_tables():
    half = D // 2
    i = np.arange(half, dtype=np.float32)
    expo = (2.0 * i / np.float32(D)).astype(np.float32)
    alpha = (1.0 / (np.float32(10000.0) ** expo)).astype(np.float32)
    ang = (np.arange(T, dtype=np.float32)[:, None] * alpha[None, :]).astype(np.float32)
    cosv = np.cos(ang).astype(np.float32).T  # [32, T]
    sinv = np.sin(ang).astype(np.float32).T
    c64 = np.concatenate([cosv, cosv], axis=0)  # [64, T]
    s64 = np.concatenate([-sinv, sinv], axis=0)
    ropeC = np.ascontiguousarray(np.concatenate([c64, c64], axis=0))  # [128, T]
    ropeS = np.ascontiguousarray(np.concatenate([s64, s64], axis=0))
    import ml_dtypes

    return ropeC.astype(ml_dtypes.bfloat16), ropeS.astype(ml_dtypes.bfloat16)


import ml_dtypes


def _round_fp32r(a):
    """Cast host data to the matmul operand dtype (bf16)."""
    return np.ascontiguousarray(
        np.asarray(a, dtype=np.float32).astype(ml_dtypes.bfloat16)
    )


def _tile128(a, KC):
    """[KC*128, N] -> [128, KC*N] with per-partition contiguous rows."""
    N = a.shape[1]
    return np.ascontiguousarray(
        a.reshape(KC, 128, N).transpose(1, 0, 2).reshape(128, KC * N)
    )


def _make_in_maps(x, wq, bq, wk, bk, wv, bv, wp, with_bias):
    ropeC, ropeS = _rope_tables()
    dmask = np.triu(np.ones((128, 128), np.float32))
    dmask2 = np.ascontiguousarray(
        np.concatenate([dmask, dmask], axis=1).astype(ml_dtypes.bfloat16)
    )
    KC = 9 if with_bias else 8
    in_maps = []
    for b in range(B):
        xb = np.ascontiguousarray(x[b].T.astype(np.float32, copy=False))  # [C, T]
        if with_bias:
            aug = np.zeros((9 * 128 - C, T), np.float32)
            aug[0, :] = 1.0
            xb = np.concatenate([xb, aug], axis=0)
        xT2 = _tile128(_round_fp32r(xb), KC)
        for g in range(2):
            sl = slice(g * CL, (g + 1) * CL)
            wqTc = np.ascontiguousarray(wq[sl, :].T.astype(np.float32, copy=False))
            wkTc = np.ascontiguousarray(wk[sl, :].T.astype(np.float32, copy=False))
            wvTc = np.ascontiguousarray(wv[sl, :].T.astype(np.float32, copy=False))
            if with_bias:
                npad = 9 * 128 - C

                def _aug_w(wT, bias):
                    a = np.zeros((npad, CL), np.float32)
                    a[0, :] = bias[sl].astype(np.float32, copy=False)
                    return np.ascontiguousarray(np.concatenate([wT, a], axis=0))

                wqTc = _aug_w(wqTc, bq)
                wkTc = _aug_w(wkTc, bk)
                wvTc = _aug_w(wvTc, bv)
            wpTc = np.ascontiguousarray(wp[:, sl].T.astype(np.float32, copy=False))
            in_maps.append(
                {
                    "xT2": xT2,
                    "wq2": _tile128(_round_fp32r(wqTc), KC),
                    "wk2": _tile128(_round_fp32r(wkTc), KC),
                    "wv2": _tile128(_round_fp32r(wvTc), KC),
                    "wp2": _tile128(_round_fp32r(wpTc), 4),
                    "ropeC": ropeC,
                    "ropeS": ropeS,
                    "dmask2": dmask2,
                }
            )
    return in_maps


def _gather(results, bp):
    out = np.empty((B, T, C), dtype=np.float32)
    bp32 = np.asarray(bp, dtype=np.float32)
    for b in range(B):
        out[b] = results[2 * b]["out"] + results[2 * b + 1]["out"] + bp32
    return out


def run(x, wq, bq, wk, bk, wv, bv, wp, bp, trace=False, **kw):
    """Build/compile (cached), run on 8 cores, gather. Returns (out, results)."""
    arrs = [np.asarray(a) for a in (x, wq, bq, wk, bk, wv, bv, wp, bp)]
    x, wq, bq, wk, bk, wv, bv, wp, bp = arrs
    with_bias = bool(np.any(bq) or np.any(bk) or np.any(bv))
    nc = _get_nc(with_bias)
    in_maps = _make_in_maps(x, wq, bq, wk, bk, wv, bv, wp, with_bias)
    res = run_bass_kernel_spmd(nc, in_maps, list(range(NCORES)), trace=trace, **kw)
    return _gather(res.results, bp), res


def kernel(x, wq, bq, wk, bk, wv, bv, wp, bp):
    out, _ = run(x, wq, bq, wk, bk, wv, bv, wp, bp)
    return out


# revision 15
# speedup vs baseline: 1.0559x; 1.0528x over previous
"""Causal self-attention (B=4, T=2048, C=1024, H=16, rope) on 8 trn2 cores.

Sharding: data-parallel over B (4) x tensor-parallel over heads (2 groups of
8 heads). Core (b, g) computes its batch's Q/K/V for its 8 heads, the full
causal attention for those heads, and a partial output projection
(y_heads @ wp_cols.T). Host sums the two head-group partials per batch and
adds the output bias.

v2: fully software-pipelined emission. The V projection runs first (only the
first 4 time-tiles are needed to start attention), then the per-head-pair
attention streams are interleaved with the NEXT pair's Q/K projection + rope
as "filler" work, so the scalar engine's exp stream (the steady-state
bottleneck) starts ~25us into the kernel instead of after all projections.
Input DMAs use host-pretiled layouts with >=4KB contiguous descriptors.
Softmax denominators use the extra all-ones column in V (row 64 of the O^T
accumulation); reciprocal uses the fast approximate DVE op. The causal mask
multiply handles both heads of a pair in one strided op.
"""

import sys

if "/opt/trn_rl_repo" not in sys.path:
    sys.path.insert(0, "/opt/trn_rl_repo")

from contextlib import ExitStack

import numpy as np

import concourse.bass as bass
import concourse.mybir as mybir
from concourse import bacc
from concourse.bass_utils import run_bass_kernel_spmd
from concourse.tile import TileContext

B, T, C = 4, 2048, 1024
H = 16
D = 64
NCORES = 8
CL = C // 2  # per-core c_out (8 heads * 64)
HL = 8  # local heads
F = mybir.dt.float32
FR = mybir.dt.bfloat16  # matmul operand dtype

_NC_CACHE = {}


def _build_nc(with_bias: bool):
    KC = 9 if with_bias else 8  # c_in chunks of 128 (one extra for bias row)
    nc = bacc.Bacc("TRN2", debug=False, num_devices=NCORES)

    xT2 = nc.declare_dram_parameter("xT2", [128, KC * T], FR, isOutput=False).ap()
    wq2 = nc.declare_dram_parameter("wq2", [128, KC * CL], FR, isOutput=False).ap()
    wk2 = nc.declare_dram_parameter("wk2", [128, KC * CL], FR, isOutput=False).ap()
    wv2 = nc.declare_dram_parameter("wv2", [128, KC * CL], FR, isOutput=False).ap()
    wp2 = nc.declare_dram_parameter("wp2", [128, 4 * C], FR, isOutput=False).ap()
    ropeC = nc.declare_dram_parameter("ropeC", [128, T], FR, isOutput=False).ap()
    ropeS = nc.declare_dram_parameter("ropeS", [128, T], FR, isOutput=False).ap()
    dmask2 = nc.declare_dram_parameter("dmask2", [128, 256], FR, isOutput=False).ap()
    out = nc.declare_dram_parameter("out", [T, C], F, isOutput=True).ap()

    EXP = mybir.ActivationFunctionType.Exp
    scale = 1.0 / float(np.sqrt(D))

    with TileContext(nc) as tc:
        with ExitStack() as ctx:
            qk_pool = ctx.enter_context(tc.tile_pool(name="qk", bufs=1))
            v_pool = ctx.enter_context(tc.tile_pool(name="v", bufs=1))
            xw_pool = ctx.enter_context(tc.tile_pool(name="xw", bufs=1))
            tpool = ctx.enter_context(tc.tile_pool(name="t1", bufs=2))
            ppool = ctx.enter_context(tc.tile_pool(name="pt", bufs=3))
            yrawp = ctx.enter_context(tc.tile_pool(name="yraw", bufs=3))
            ytmpp = ctx.enter_context(tc.tile_pool(name="ytmp", bufs=2))
            ynp = ctx.enter_context(tc.tile_pool(name="yn", bufs=1))
            osbp = ctx.enter_context(tc.tile_pool(name="osb", bufs=3))
            dpool = ctx.enter_context(tc.tile_pool(name="dd", bufs=2))
            bcpool = ctx.enter_context(tc.tile_pool(name="bc", bufs=3))
            spool = ctx.enter_context(tc.tile_pool(name="sps", bufs=2, space="PSUM"))
            opool = ctx.enter_context(tc.tile_pool(name="ops", bufs=1, space="PSUM"))
            prpool = ctx.enter_context(tc.tile_pool(name="prs", bufs=2, space="PSUM"))

            qt_sb = [
                qk_pool.tile([128, T], FR, tag=f"qt{m}", name=f"qt{m}")
                for m in range(4)
            ]
            kt_sb = [
                qk_pool.tile([128, T], FR, tag=f"kt{m}", name=f"kt{m}")
                for m in range(4)
            ]
            vaug = [
                v_pool.tile([128, HL, D + 1], FR, tag=f"va{j}", name=f"va{j}")
                for j in range(16)
            ]

            x_sb = xw_pool.tile([128, KC, T], FR, tag="x", name="x")
            wq_sb = xw_pool.tile([128, KC, CL], FR, tag="wq", name="wq")
            wk_sb = xw_pool.tile([128, KC, CL], FR, tag="wk", name="wk")
            wv_sb = xw_pool.tile([128, KC, CL], FR, tag="wv", name="wv")
            wp_sb = xw_pool.tile([128, 4, C], FR, tag="wp", name="wp")
            rc_sb = xw_pool.tile([128, T], FR, tag="rc", name="rc")
            rs_sb = xw_pool.tile([128, T], FR, tag="rs", name="rs")
            dm_sb = xw_pool.tile([128, 256], FR, tag="dm", name="dm")

            # ---- input DMAs, need-ordered: V deps first ----
            nc.sync.dma_start(out=wv_sb, in_=wv2.rearrange("p (k m) -> p k m", k=KC))
            x_r = xT2.rearrange("p (k t) -> p k t", k=KC)
            for k in range(KC):
                nc.sync.dma_start(out=x_sb[:, k, :], in_=x_r[:, k, :])
            nc.sync.dma_start(out=wk_sb, in_=wk2.rearrange("p (k m) -> p k m", k=KC))
            nc.sync.dma_start(out=wq_sb, in_=wq2.rearrange("p (k m) -> p k m", k=KC))
            nc.sync.dma_start(out=rc_sb, in_=ropeC)
            nc.sync.dma_start(out=rs_sb, in_=ropeS)
            nc.sync.dma_start(out=dm_sb, in_=dmask2)
            nc.sync.dma_start(out=wp_sb, in_=wp2.rearrange("p (j n) -> p j n", j=4))

            # ones column of V-augmented tiles (softmax denominators)
            for j in range(16):
                nc.gpsimd.memset(vaug[j][:, :, D : D + 1], 1.0)

            # preload the exp table set during the input DMA window
            warm = tpool.tile([128, 512], FR, tag="qcp", name="warm")
            nc.gpsimd.memset(warm[0:1, 0:1], 0.0)
            nc.scalar.activation(warm[0:1, 1:2], warm[0:1, 0:1], EXP)

            # ---------------- filler work-unit generators ----------------
            def v_group(jj):
                ps = prpool.tile([128, 512], F, tag="pr", name="pr")
                for k in range(KC):
                    nc.tensor.matmul(
                        ps,
                        lhsT=x_sb[:, k, 128 * jj : 128 * (jj + 1)],
                        rhs=wv_sb[:, k, :],
                        start=(k == 0),
                        stop=(k == KC - 1),
                    )
                    if k == 3:
                        yield
                nc.vector.tensor_copy(
                    out=vaug[jj][:, :, 0:D],
                    in_=ps.rearrange("p (h d) -> p h d", h=HL),
                )
                yield

            def qk_group(m, which, t):
                wsb = wk_sb if which == "k" else wq_sb
                dst = kt_sb[m] if which == "k" else qt_sb[m]
                ps = prpool.tile([128, 512], F, tag="pr", name="pr")
                for k in range(KC):
                    nc.tensor.matmul(
                        ps,
                        lhsT=wsb[:, k, 128 * m : 128 * (m + 1)],
                        rhs=x_sb[:, k, 512 * t : 512 * (t + 1)],
                        start=(k == 0),
                        stop=(k == KC - 1),
                    )
                    if k == 3:
                        yield
                qcp = tpool.tile([128, 512], FR, tag="qcp", name="qcp")
                nc.vector.tensor_copy(qcp, ps)
                qsw = tpool.tile([128, 512], FR, tag="qsw", name="qsw")
                for a, b in ((0, 32), (32, 0), (64, 96), (96, 64)):
                    nc.sync.dma_start(out=qsw[a : a + 32, :], in_=qcp[b : b + 32, :])
                yield
                t1 = tpool.tile([128, 512], FR, tag="t1", name="t1")
                t2 = tpool.tile([128, 512], FR, tag="t2", name="t2")
                nc.gpsimd.tensor_mul(t1, qcp, rc_sb[:, 512 * t : 512 * (t + 1)])
                nc.vector.tensor_mul(t2, qsw, rs_sb[:, 512 * t : 512 * (t + 1)])
                nc.vector.tensor_add(dst[:, 512 * t : 512 * (t + 1)], t1, t2)
                yield

            def outproj_group(ci, yn):
                for g in range(8):
                    tt, cc = g % 4, g // 4
                    pr = prpool.tile([128, 512], F, tag="pr", name="pr")
                    for p4 in range(4):
                        nc.tensor.matmul(
                            pr,
                            lhsT=yn[p4][:, 128 * tt : 128 * (tt + 1)],
                            rhs=wp_sb[:, p4, 512 * cc : 512 * (cc + 1)],
                            start=(p4 == 0),
                            stop=(p4 == 3),
                        )
                    osb = osbp.tile([128, 512], F, tag="osb", name="osb")
                    nc.vector.tensor_copy(osb, pr)
                    nc.sync.dma_start(
                        out=out[
                            512 * ci + 128 * tt : 512 * ci + 128 * (tt + 1),
                            512 * cc : 512 * (cc + 1),
                        ],
                        in_=osb,
                    )
                    yield

            # ordered filler queue: (key, generator); ensure() drains in order
            fill_q = []
            fill_done = set()

            def push(key, gen):
                fill_q.append((key, gen))

            def ensure(key):
                while key not in fill_done and fill_q:
                    k0, g0 = fill_q[0]
                    try:
                        next(g0)
                    except StopIteration:
                        fill_done.add(k0)
                        fill_q.pop(0)

            def pump(n):
                for _ in range(n):
                    if not fill_q:
                        return
                    k0, g0 = fill_q[0]
                    try:
                        next(g0)
                    except StopIteration:
                        fill_done.add(k0)
                        fill_q.pop(0)

            # eager prologue: V tiles 0-3 and the (m=0, t=0) Q/K chunks
            for jj in range(4):
                g = v_group(jj)
                for _ in g:
                    pass
                fill_done.add(("v", jj))
            for which in ("k", "q"):
                g = qk_group(0, which, 0)
                for _ in g:
                    pass
                fill_done.add((which, 0, 0))

            # filler queue in need-order for the p=0 attention sweep, then
            # the later pairs' projections
            for ci in range(1, 4):
                for jj in range(4 * ci, 4 * ci + 4):
                    push(("v", jj), v_group(jj))
                push(("k", 0, ci), qk_group(0, "k", ci))
                push(("q", 0, ci), qk_group(0, "q", ci))
            for m in range(1, 4):
                for t in range(4):
                    push(("k", m, t), qk_group(m, "k", t))
                    push(("q", m, t), qk_group(m, "q", t))

            # ---------------- attention sweeps, pair-major ----------------
            yn_store = {}  # ci -> [yn tiles p=0..3]
            ep_q = []  # pipelined epilogue records [yn, yraw, d_sb, bc, ci, p]
            outproj_ready = []  # list of ci whose outproj can be pushed

            def flush_norm(e):
                pyn, pyraw, _, pbc, pci, pp = e
                pynorm = ynp.tile(
                    [128, 512], FR, tag=f"yn{pci}_{pp}", name=f"yn{pci}_{pp}"
                )
                nc.vector.tensor_mul(pynorm, pyraw, pbc)
                pyn.append(pynorm)
                if pp == 3:
                    outproj_ready.append(pci)

            for p in range(4):
                for ci in range(4):
                    ensure(("v", 4 * ci + 3))
                    ensure(("k", p, ci))
                    ensure(("q", p, ci))
                    yn = yn_store.setdefault(ci, [])
                    o_ps = [
                        opool.tile([128, 512], F, tag=f"o{h}", name=f"o{h}")
                        for h in range(2)
                    ]
                    ntj = 4 * ci + 4
                    for tj in range(ntj):
                        kk = tj - 4 * ci
                        off = 128 * max(kk, 0)
                        s_ps = spool.tile([128, 1024], F, tag="s", name="s")
                        for h in range(2):
                            nc.tensor.matmul(
                                s_ps[:, 512 * h + off : 512 * h + 512],
                                lhsT=kt_sb[p][
                                    64 * h : 64 * h + 64,
                                    128 * tj : 128 * (tj + 1),
                                ],
                                rhs=qt_sb[p][
                                    64 * h : 64 * h + 64,
                                    512 * ci + off : 512 * (ci + 1),
                                ],
                                start=True,
                                stop=True,
                                tile_position=(64 * h, 0),
                            )
                        pt = ppool.tile([128, 1024], FR, tag="pt", name="pt")
                        if kk < 0:
                            nc.scalar.activation(pt, s_ps, EXP, scale=scale)
                        else:
                            s_v = s_ps.rearrange("q (h n) -> q h n", h=2)[:, :, off:]
                            p_v = pt.rearrange("q (h n) -> q h n", h=2)[:, :, off:]
                            nc.scalar.activation(p_v, s_v, EXP, scale=scale)
                            # multiplicative causal mask on both diagonal blocks
                            blk = pt.rearrange("q (h n) -> q h n", h=2)[
                                :, :, off : off + 128
                            ]
                            nc.gpsimd.tensor_mul(
                                blk, blk, dm_sb.rearrange("q (h n) -> q h n", h=2)
                            )
                        for h in range(2):
                            nc.tensor.matmul(
                                o_ps[h][0 : D + 1, off:512],
                                lhsT=vaug[tj][:, 2 * p + h, :],
                                rhs=pt[:, 512 * h + off : 512 * h + 512],
                                start=(tj == 0),
                                stop=(tj == ntj - 1),
                                skip_group_check=True,
                            )
                        pump(2)

                    # epilogue: extract O + denominators. The normalization
                    # chain is pipelined 3 sweeps deep (copies+saves now,
                    # recip+broadcast next sweep, normalize the sweep after)
                    # so its DMA drains never gate the attention steady state.
                    yraw = yrawp.tile([128, 512], F, tag="yraw", name="yraw")
                    ytmp = ytmpp.tile([128, 512], F, tag="ytmp", name="ytmp")
                    d_sb = dpool.tile([128, 1024], F, tag="D", name="D")
                    nc.vector.tensor_copy(yraw[0:65, :], o_ps[0][0:65, :])
                    nc.vector.tensor_copy(ytmp[0:65, :], o_ps[1][0:65, :])
                    nc.gpsimd.dma_start(out=d_sb[0:1, 0:512], in_=yraw[64:65, :])
                    nc.gpsimd.dma_start(out=d_sb[1:2, 0:512], in_=ytmp[64:65, :])
                    nc.gpsimd.dma_start(out=yraw[64:128, :], in_=ytmp[0:64, :])
                    ep_q.append([yn, yraw, d_sb, None, ci, p])
                    if len(ep_q) >= 2:
                        e = ep_q[-2]
                        nc.vector.reciprocal_approx_fast(
                            out=e[2][0:2, 512:1024], in_=e[2][0:2, 0:512]
                        )
                        bc = bcpool.tile([128, 512], F, tag="bc", name="bc")
                        for h in range(2):
                            sl = e[2][h : h + 1, 512:1024]
                            bsrc = bass.AP(
                                tensor=sl.tensor,
                                offset=sl.offset,
                                ap=[list(sl.ap[0]), [0, 64], [1, 512]],
                            )
                            nc.gpsimd.dma_start(
                                out=bc[64 * h : 64 * h + 64, :], in_=bsrc
                            )
                        e[3] = bc
                    if len(ep_q) >= 3:
                        flush_norm(ep_q.pop(0))
                    # during the last pair, stream out-projection as filler
                    while outproj_ready:
                        pci = outproj_ready.pop(0)
                        push(("op", pci), outproj_group(pci, yn_store[pci]))

            # tail: flush the pipelined epilogues, remaining out-projection
            e = ep_q[-1]
            nc.vector.reciprocal_approx_fast(
                out=e[2][0:2, 512:1024], in_=e[2][0:2, 0:512]
            )
            bc = bcpool.tile([128, 512], F, tag="bc", name="bc")
            for h in range(2):
                sl = e[2][h : h + 1, 512:1024]
                bsrc = bass.AP(
                    tensor=sl.tensor,
                    offset=sl.offset,
                    ap=[list(sl.ap[0]), [0, 64], [1, 512]],
                )
                nc.gpsimd.dma_start(out=bc[64 * h : 64 * h + 64, :], in_=bsrc)
            e[3] = bc
            while ep_q:
                flush_norm(ep_q.pop(0))
                while outproj_ready:
                    pci2 = outproj_ready.pop(0)
                    push(("op", pci2), outproj_group(pci2, yn_store[pci2]))
                pump(10**6)
            pump(10**6)

    nc.compile()
    return nc


def _get_nc(with_bias: bool):
    if with_bias not in _NC_CACHE:
        _NC_CACHE[with_bias] = _build_nc(with_bias)
    return _NC_CACHE[with_bias]


def _rope_tables():
    half = D // 2
    i = np.arange(half, dtype=np.float32)
    expo = (2.0 * i / np.float32(D)).astype(np.float32)
    alpha = (1.0 / (np.float32(10000.0) ** expo)).astype(np.float32)
    ang = (np.arange(T, dtype=np.float32)[:, None] * alpha[None, :]).astype(np.float32)
    cosv = np.cos(ang).astype(np.float32).T  # [32, T]
    sinv = np.sin(ang).astype(np.float32).T
    c64 = np.concatenate([cosv, cosv], axis=0)  # [64, T]
    s64 = np.concatenate([-sinv, sinv], axis=0)
    ropeC = np.ascontiguousarray(np.concatenate([c64, c64], axis=0))  # [128, T]
    ropeS = np.ascontiguousarray(np.concatenate([s64, s64], axis=0))
    import ml_dtypes

    return ropeC.astype(ml_dtypes.bfloat16), ropeS.astype(ml_dtypes.bfloat16)


import ml_dtypes


def _round_fp32r(a):
    """Cast host data to the matmul operand dtype (bf16)."""
    return np.ascontiguousarray(
        np.asarray(a, dtype=np.float32).astype(ml_dtypes.bfloat16)
    )


def _tile128(a, KC):
    """[KC*128, N] -> [128, KC*N] with per-partition contiguous rows."""
    N = a.shape[1]
    return np.ascontiguousarray(
        a.reshape(KC, 128, N).transpose(1, 0, 2).reshape(128, KC * N)
    )


def _make_in_maps(x, wq, bq, wk, bk, wv, bv, wp, with_bias):
    ropeC, ropeS = _rope_tables()
    dmask = np.triu(np.ones((128, 128), np.float32))
    dmask2 = np.ascontiguousarray(
        np.concatenate([dmask, dmask], axis=1).astype(ml_dtypes.bfloat16)
    )
    KC = 9 if with_bias else 8
    in_maps = []
    for b in range(B):
        xb = np.ascontiguousarray(x[b].T.astype(np.float32, copy=False))  # [C, T]
        if with_bias:
            aug = np.zeros((9 * 128 - C, T), np.float32)
            aug[0, :] = 1.0
            xb = np.concatenate([xb, aug], axis=0)
        xT2 = _tile128(_round_fp32r(xb), KC)
        for g in range(2):
            sl = slice(g * CL, (g + 1) * CL)
            wqTc = np.ascontiguousarray(wq[sl, :].T.astype(np.float32, copy=False))
            wkTc = np.ascontiguousarray(wk[sl, :].T.astype(np.float32, copy=False))
            wvTc = np.ascontiguousarray(wv[sl, :].T.astype(np.float32, copy=False))
            if with_bias:
                npad = 9 * 128 - C

                def _aug_w(wT, bias):
                    a = np.zeros((npad, CL), np.float32)
                    a[0, :] = bias[sl].astype(np.float32, copy=False)
                    return np.ascontiguousarray(np.concatenate([wT, a], axis=0))

                wqTc = _aug_w(wqTc, bq)
                wkTc = _aug_w(wkTc, bk)
                wvTc = _aug_w(wvTc, bv)
            wpTc = np.ascontiguousarray(wp[:, sl].T.astype(np.float32, copy=False))
            in_maps.append(
                {
                    "xT2": xT2,
                    "wq2": _tile128(_round_fp32r(wqTc), KC),
                    "wk2": _tile128(_round_fp32r(wkTc), KC),
                    "wv2": _tile128(_round_fp32r(wvTc), KC),
                    "wp2": _tile128(_round_fp32r(wpTc), 4),
                    "ropeC": ropeC,
                    "ropeS": ropeS,
                    "dmask2": dmask2,
                }
            )
    return in_maps


def _gather(results, bp):
    out = np.empty((B, T, C), dtype=np.float32)
    bp32 = np.asarray(bp, dtype=np.float32)
    for b in range(B):
        out[b] = results[2 * b]["out"] + results[2 * b + 1]["out"] + bp32
    return out


def run(x, wq, bq, wk, bk, wv, bv, wp, bp, trace=False, **kw):
    """Build/compile (cached), run on 8 cores, gather. Returns (out, results)."""
    arrs = [np.asarray(a) for a in (x, wq, bq, wk, bk, wv, bv, wp, bp)]
    x, wq, bq, wk, bk, wv, bv, wp, bp = arrs
    with_bias = bool(np.any(bq) or np.any(bk) or np.any(bv))
    nc = _get_nc(with_bias)
    in_maps = _make_in_maps(x, wq, bq, wk, bk, wv, bv, wp, with_bias)
    res = run_bass_kernel_spmd(nc, in_maps, list(range(NCORES)), trace=trace, **kw)
    return _gather(res.results, bp), res


def kernel(x, wq, bq, wk, bk, wv, bv, wp, bp):
    out, _ = run(x, wq, bq, wk, bk, wv, bv, wp, bp)
    return out
